# revision 7
# baseline (speedup 1.0000x reference)
"""Trainium2 Bass kernel for nn_HashEncoder (instant-NGP style hash-grid encoder).

Contract: kernel(inputs, embeddings) -> [1M, 32] f32.
Sharding: data-parallel over points, 8 cores; full 57MB table in each core's HBM.
Device work per core: normalize coords, per level compute corner hash indices on
DVE (uint32 math, 16-bit decomposed multiplies — DVE uint32 mult saturates, but
only the low 19 hash bits are needed), gather 8 corner rows per point via
indirect DMA ([128,1]-offset form, one corner x 128 points per instruction),
trilinear-interpolate via broadcast-AP multiplies + segmented tensor_reduce.
"""
import sys

if "/opt/trn_rl_repo" not in sys.path:
    sys.path.insert(0, "/opt/trn_rl_repo")

import numpy as np

# ---- problem constants (hardcoded per harness contract) ----
D, L, C, H = 3, 16, 2, 16
T = 2 ** 19
BOUND = 1.0
PRIMES = (1, 2654435761, 805459861)
B_FULL = 1_000_000
N_CORES = 8


def _make_offsets():
    offs, o = [0], 0
    for l in range(L):
        res = H * (2 ** l)
        o += min(T, (res + 1) ** D)
        offs.append(o)
    return offs


OFFSETS = _make_offsets()
N_PARAMS = OFFSETS[-1]  # 7131219

# per-core point layout: NPC points = 128 partitions x NC cols, point(p, j) = p*NC + j
NC_COLS = 977
NPC = 128 * NC_COLS          # 125056
B_PAD = NPC * N_CORES        # 1000448
CN_TILE = 192                # cols per SBUF tile
MASK19 = 0x7FFFF


def _build(npc, nc_cols, cn_tile, levels):
    import concourse.bass as bass
    import concourse.tile as tile
    from concourse import bacc, mybir

    dt = mybir.dt
    Alu = mybir.AluOpType
    P = 128

    nc = bacc.Bacc("TRN2", target_bir_lowering=False, debug=False,
                   enable_asserts=False, num_devices=N_CORES)
    pts_d = nc.dram_tensor("pts", [npc, 3], dt.float32, kind="ExternalInput")
    emb_d = nc.dram_tensor("emb", [N_PARAMS, C], dt.float32, kind="ExternalInput")
    nout = 2 * len(levels)
    out_d = nc.dram_tensor("out", [npc, nout], dt.float32, kind="ExternalOutput")

    pts_v = pts_d.ap().rearrange("(p n) d -> p n d", p=P)   # [128, nc_cols, 3]
    out_v = out_d.ap().rearrange("(p n) c -> p n c", p=P)   # [128, nc_cols, nout]

    col_tiles = []
    jb = 0
    while jb < nc_cols:
        cn = min(cn_tile, nc_cols - jb)
        col_tiles.append((jb, cn))
        jb += cn

    with tile.TileContext(nc) as tc:
        with tc.tile_pool(name="sb", bufs=2) as sb, \
             tc.tile_pool(name="consts", bufs=1) as cpool:

            # uint32 constant tiles [P, 1] (int immediates aren't supported)
            _consts = {}

            def cu(val):
                if val not in _consts:
                    t = cpool.tile([P, 1], dt.uint32, tag=f"c{val}")
                    nc.vector.memset(t[:, :], val)
                    _consts[val] = t
                return _consts[val][:, :1]

            def ibc(val, shape_free):
                # broadcast [P,1] uint32 const along free dims
                return cu(val).to_broadcast([P] + shape_free)


            for ti, (jb, cn) in enumerate(col_tiles):
                pts_t = sb.tile([P, cn, 3], dt.float32, tag="pts")
                nc.sync.dma_start(out=pts_t[:, :, :], in_=pts_v[:, jb:jb + cn, :])

                # xn = clip((pts+1)*0.5, 0, 1)
                xn = sb.tile([P, cn, 3], dt.float32, tag="xn")
                nc.vector.tensor_scalar(
                    out=xn[:, :, :], in0=pts_t[:, :, :], scalar1=0.5, scalar2=0.5,
                    op0=Alu.mult, op1=Alu.add)
                nc.vector.tensor_scalar(
                    out=xn[:, :, :], in0=xn[:, :, :], scalar1=1.0, scalar2=0.0,
                    op0=Alu.min, op1=Alu.max)

                outt = sb.tile([P, cn, nout], dt.float32, tag="outt")

                for li, l in enumerate(levels):
                    res = H * (2 ** l)
                    size = OFFSETS[l + 1] - OFFSETS[l]
                    dense = (res + 1) ** D <= size

                    pos3 = sb.tile([P, cn, 3], dt.float32, tag="pos3")
                    nc.vector.tensor_scalar(
                        out=pos3[:, :, :], in0=xn[:, :, :], scalar1=float(res),
                        scalar2=None, op0=Alu.mult)

                    # floor: r = rint(pos); rf = f32(r); gt = (rf > pos); pgf = rf-gt
                    pgu = sb.tile([P, cn, 3], dt.uint32, tag="pgu")
                    rf = sb.tile([P, cn, 3], dt.float32, tag="rf")
                    gt = sb.tile([P, cn, 3], dt.float32, tag="gtf")
                    nc.vector.tensor_copy(out=pgu[:, :, :], in_=pos3[:, :, :])
                    nc.vector.tensor_copy(out=rf[:, :, :], in_=pgu[:, :, :])
                    nc.vector.tensor_tensor(
                        out=gt[:, :, :], in0=rf[:, :, :], in1=pos3[:, :, :],
                        op=Alu.is_gt)
                    nc.vector.tensor_tensor(
                        out=rf[:, :, :], in0=rf[:, :, :], in1=gt[:, :, :],
                        op=Alu.subtract)
                    nc.vector.tensor_scalar(
                        out=rf[:, :, :], in0=rf[:, :, :], scalar1=float(res - 1),
                        scalar2=None, op0=Alu.min)
                    # f2[0]=1-frac, f2[1]=frac ; frac = pos - pgf
                    f2 = sb.tile([P, 2, 3, cn], dt.float32, tag="f2")
                    frac = sb.tile([P, cn, 3], dt.float32, tag="frac")
                    nc.vector.tensor_tensor(
                        out=frac[:, :, :], in0=pos3[:, :, :], in1=rf[:, :, :],
                        op=Alu.subtract)
                    nc.vector.tensor_copy(out=pgu[:, :, :], in_=rf[:, :, :])
                    for d in range(3):
                        nc.vector.tensor_copy(
                            out=f2[:, 1, d, :], in_=frac[:, :, d])
                        # (frac * -1) - (-1) = 1 - frac
                        nc.vector.tensor_scalar(
                            out=f2[:, 0, d, :], in0=frac[:, :, d], scalar1=-1.0,
                            scalar2=-1.0, op0=Alu.mult, op1=Alu.subtract)

                    # ---- corner term pairs trm[d][0/1]: [P, cn] uint32 ----
                    trm = sb.tile([P, 3, 2, cn], dt.uint32, tag="trm")
                    nc.vector.tensor_copy(out=trm[:, 0, 0, :], in_=pgu[:, :, 0])
                    nc.vector.tensor_tensor(
                        out=trm[:, 0, 1, :], in0=pgu[:, :, 0], in1=ibc(1, [cn]),
                        op=Alu.add)
                    if dense:
                        s1, s2 = res + 1, (res + 1) ** 2
                        for d, s in ((1, s1), (2, s2)):
                            nc.vector.tensor_tensor(
                                out=trm[:, d, 0, :], in0=pgu[:, :, d],
                                in1=ibc(s, [cn]), op=Alu.mult)
                            nc.vector.tensor_tensor(
                                out=trm[:, d, 1, :], in0=trm[:, d, 0, :],
                                in1=ibc(s, [cn]), op=Alu.add)
                    else:
                        # y*p mod 2^19 via 5-bit chunks: products < 2^24 stay
                        # exact through the DVE's float multiply path.
                        nbits = l + 5
                        nch = -(-nbits // 5)
                        for d in (1, 2):
                            p = PRIMES[d]
                            acc = None
                            for jc in range(nch):
                                pk = (p << (5 * jc)) & MASK19
                                nib = sb.tile([P, cn], dt.uint32, tag="nib")
                                if jc == 0:
                                    nc.vector.tensor_scalar(
                                        out=nib[:, :], in0=pgu[:, :, d],
                                        scalar1=cu(31), scalar2=None,
                                        op0=Alu.bitwise_and)
                                else:
                                    nc.vector.tensor_tensor(
                                        out=nib[:, :], in0=pgu[:, :, d],
                                        in1=ibc(5 * jc, [cn]),
                                        op=Alu.logical_shift_right)
                                    nc.vector.tensor_scalar(
                                        out=nib[:, :], in0=nib[:, :],
                                        scalar1=cu(31), scalar2=None,
                                        op0=Alu.bitwise_and)
                                nc.vector.tensor_tensor(
                                    out=nib[:, :], in0=nib[:, :],
                                    in1=ibc(pk, [cn]), op=Alu.mult)
                                nc.vector.tensor_scalar(
                                    out=nib[:, :], in0=nib[:, :],
                                    scalar1=cu(MASK19), scalar2=None,
                                    op0=Alu.bitwise_and)
                                if acc is None:
                                    acc = sb.tile([P, cn], dt.uint32, tag="hacc")
                                    nc.vector.tensor_copy(out=acc[:, :], in_=nib[:, :])
                                else:
                                    nc.vector.tensor_tensor(
                                        out=acc[:, :], in0=acc[:, :],
                                        in1=nib[:, :], op=Alu.add)
                            nc.vector.tensor_copy(out=trm[:, d, 0, :], in_=acc[:, :])
                            nc.vector.tensor_tensor(
                                out=trm[:, d, 1, :], in0=trm[:, d, 0, :],
                                in1=ibc(p & MASK19, [cn]), op=Alu.add)

                    # ---- combine to 8 corner indices (k = bx*4 + by*2 + bz) ----
                    comb_op = Alu.add if dense else Alu.bitwise_xor
                    trm_f = trm[:, :, :, :]
                    part = trm_f.ap[0]
                    xy = sb.tile([P, 2, 2, cn], dt.uint32, tag="xy")
                    in_x = bass.AP(trm_f.tensor, trm[:, 0, 0, :].offset,
                                   [part, [cn, 2], [0, 2], [1, cn]])
                    in_y = bass.AP(trm_f.tensor, trm[:, 1, 0, :].offset,
                                   [part, [0, 2], [cn, 2], [1, cn]])
                    nc.vector.tensor_tensor(
                        out=xy[:, :, :, :], in0=in_x, in1=in_y, op=comb_op)
                    idx8 = sb.tile([P, 8, cn], dt.uint32, tag="idx8")
                    xy_f = xy[:, :, :, :]
                    idx8_f = idx8[:, :, :]
                    for bz in range(2):
                        in_xy = bass.AP(xy_f.tensor, xy_f.offset,
                                        [xy_f.ap[0], [2 * cn, 2], [cn, 2],
                                         [1, cn]])
                        in_z = bass.AP(trm_f.tensor,
                                       trm[:, 2, bz, :].offset,
                                       [part, [0, 2], [0, 2], [1, cn]])
                        o_z = bass.AP(idx8_f.tensor, idx8_f.offset + bz * cn,
                                      [idx8_f.ap[0], [4 * cn, 2], [2 * cn, 2],
                                       [1, cn]])
                        nc.vector.tensor_tensor(
                            out=o_z, in0=in_xy, in1=in_z, op=comb_op)
                    if not dense:
                        nc.vector.tensor_scalar(
                            out=idx8[:, :, :], in0=idx8[:, :, :],
                            scalar1=cu(MASK19), scalar2=None,
                            op0=Alu.bitwise_and)
                    nc.vector.tensor_tensor(
                        out=idx8[:, :, :], in0=idx8[:, :, :],
                        in1=ibc(OFFSETS[l], [8, cn]), op=Alu.add)

                    # ---- gathers: one [128,1]-offset indirect DMA per (corner, col)
                    feats = sb.tile([P, 8, cn, 2], dt.float32, tag="feats")
                    idx8_i = idx8[:, :, :].bitcast(dt.int32)
                    for k in range(8):
                        for j in range(cn):
                            nc.gpsimd.indirect_dma_start(
                                out=feats[:, k, j, :],
                                out_offset=None,
                                in_=emb_d[:, :],
                                in_offset=bass.IndirectOffsetOnAxis(
                                    ap=idx8_i[:, k, j:j + 1], axis=0),
                            )

                    # ---- weights: w8[k] = fx_bx * fy_by * fz_bz ----
                    f2_f = f2[:, :, :, :]
                    xyw = sb.tile([P, 2, 2, cn], dt.float32, tag="xyw")
                    wx = bass.AP(f2_f.tensor, f2[:, 0, 0, :].offset,
                                 [f2_f.ap[0], [3 * cn, 2], [0, 2], [1, cn]])
                    wy = bass.AP(f2_f.tensor, f2[:, 0, 1, :].offset,
                                 [f2_f.ap[0], [0, 2], [3 * cn, 2], [1, cn]])
                    nc.vector.tensor_tensor(
                        out=xyw[:, :, :, :], in0=wx, in1=wy, op=Alu.mult)
                    w8 = sb.tile([P, 8, cn], dt.float32, tag="w8")
                    xyw_f = xyw[:, :, :, :]
                    w8_f = w8[:, :, :]
                    for bz in range(2):
                        in_xyw = bass.AP(xyw_f.tensor, xyw_f.offset,
                                         [xyw_f.ap[0], [2 * cn, 2], [cn, 2],
                                          [1, cn]])
                        wz = bass.AP(f2_f.tensor,
                                     f2[:, bz, 2, :].offset,
                                     [f2_f.ap[0], [0, 2], [0, 2], [1, cn]])
                        o_w = bass.AP(w8_f.tensor, w8_f.offset + bz * cn,
                                      [w8_f.ap[0], [4 * cn, 2], [2 * cn, 2],
                                       [1, cn]])
                        nc.vector.tensor_tensor(out=o_w, in0=in_xyw, in1=wz,
                                                op=Alu.mult)

                    # ---- interp: per channel, prod = w8*feats_c ; reduce over k
                    feats_f = feats[:, :, :, :]
                    outt_f = outt[:, :, :]
                    for c in range(2):
                        prod = sb.tile([P, cn, 8], dt.float32, tag="prod")
                        w_v = bass.AP(w8_f.tensor, w8_f.offset,
                                      [w8_f.ap[0], [1, cn], [cn, 8]])
                        f_v = bass.AP(feats_f.tensor, feats_f.offset + c,
                                      [feats_f.ap[0], [2, cn], [2 * cn, 8]])
                        nc.vector.tensor_tensor(
                            out=prod[:, :, :], in0=w_v, in1=f_v, op=Alu.mult)
                        res_v = bass.AP(outt_f.tensor,
                                        outt_f.offset + li * 2 + c,
                                        [outt_f.ap[0], [nout, cn]])
                        nc.vector.tensor_reduce(
                            out=res_v, in_=prod[:, :, :],
                            axis=mybir.AxisListType.X, op=Alu.add)

                nc.sync.dma_start(out=out_v[:, jb:jb + cn, :], in_=outt[:, :, :])

    nc.compile()
    return nc


_BUILD_CACHE = {}


def _get_nc(npc, nc_cols, cn_tile, levels):
    key = (npc, nc_cols, cn_tile, tuple(levels))
    if key not in _BUILD_CACHE:
        _BUILD_CACHE[key] = _build(npc, nc_cols, cn_tile, levels)
    return _BUILD_CACHE[key]


def kernel(inputs: np.ndarray, embeddings: np.ndarray, _trace=False) -> np.ndarray:
    from concourse.bass_utils import run_bass_kernel_spmd

    inputs = np.ascontiguousarray(inputs, dtype=np.float32)
    embeddings = np.ascontiguousarray(embeddings, dtype=np.float32)
    B = inputs.shape[0]

    pts_pad = np.zeros((B_PAD, 3), dtype=np.float32)
    pts_pad[:B] = inputs
    nc = _get_nc(NPC, NC_COLS, CN_TILE, list(range(L)))
    in_maps = [dict(pts=pts_pad[c * NPC:(c + 1) * NPC], emb=embeddings)
               for c in range(N_CORES)]
    import time as _time
    _t0 = _time.time()
    r = run_bass_kernel_spmd(nc, in_maps, core_ids=list(range(N_CORES)),
                             trace=False)
    kernel._last_wall_s = _time.time() - _t0
    out = np.concatenate([r.results[c]["out"] for c in range(N_CORES)], axis=0)
    kernel._last_exec_ns = r.exec_time_ns
    return out[:B]


# revision 8
# speedup vs baseline: 1.0867x; 1.0867x over previous
"""Trainium2 Bass kernel for nn_HashEncoder (instant-NGP style hash-grid encoder).

Contract: kernel(inputs, embeddings) -> [1M, 32] f32.
Sharding: data-parallel over points, 8 cores; full 57MB table in each core's HBM.
Device work per core: normalize coords, per level compute corner hash indices on
DVE (uint32 math, 16-bit decomposed multiplies — DVE uint32 mult saturates, but
only the low 19 hash bits are needed), gather 8 corner rows per point via
indirect DMA ([128,1]-offset form, one corner x 128 points per instruction),
trilinear-interpolate via broadcast-AP multiplies + segmented tensor_reduce.
"""
import sys

if "/opt/trn_rl_repo" not in sys.path:
    sys.path.insert(0, "/opt/trn_rl_repo")

import numpy as np

# ---- problem constants (hardcoded per harness contract) ----
D, L, C, H = 3, 16, 2, 16
T = 2 ** 19
BOUND = 1.0
PRIMES = (1, 2654435761, 805459861)
B_FULL = 1_000_000
N_CORES = 8


def _make_offsets():
    offs, o = [0], 0
    for l in range(L):
        res = H * (2 ** l)
        o += min(T, (res + 1) ** D)
        offs.append(o)
    return offs


OFFSETS = _make_offsets()
N_PARAMS = OFFSETS[-1]  # 7131219

# per-core point layout: NPC points = 128 partitions x NC cols, point(p, j) = p*NC + j
NC_COLS = 977
NPC = 128 * NC_COLS          # 125056
B_PAD = NPC * N_CORES        # 1000448
CN_TILE = 192                # cols per SBUF tile
MASK19 = 0x7FFFF


def _build(npc, nc_cols, cn_tile, levels):
    import concourse.bass as bass
    import concourse.tile as tile
    from concourse import bacc, mybir

    dt = mybir.dt
    Alu = mybir.AluOpType
    P = 128

    nc = bacc.Bacc("TRN2", target_bir_lowering=False, debug=False,
                   enable_asserts=False, num_devices=N_CORES)
    pts_d = nc.dram_tensor("pts", [npc, 3], dt.float32, kind="ExternalInput")
    emb_d = nc.dram_tensor("emb", [N_PARAMS, C], dt.float32, kind="ExternalInput")
    nout = 2 * len(levels)
    out_d = nc.dram_tensor("out", [npc, nout], dt.float32, kind="ExternalOutput")

    pts_v = pts_d.ap().rearrange("(p n) d -> p n d", p=P)   # [128, nc_cols, 3]
    out_v = out_d.ap().rearrange("(p n) c -> p n c", p=P)   # [128, nc_cols, nout]

    col_tiles = []
    jb = 0
    while jb < nc_cols:
        cn = min(cn_tile, nc_cols - jb)
        col_tiles.append((jb, cn))
        jb += cn

    with tile.TileContext(nc) as tc:
        with tc.tile_pool(name="sb", bufs=2) as sb, \
             tc.tile_pool(name="consts", bufs=1) as cpool:

            # uint32 constant tiles [P, 1] (int immediates aren't supported)
            _consts = {}

            def cu(val):
                if val not in _consts:
                    t = cpool.tile([P, 1], dt.uint32, tag=f"c{val}")
                    nc.vector.memset(t[:, :], val)
                    _consts[val] = t
                return _consts[val][:, :1]

            def ibc(val, shape_free):
                # broadcast [P,1] uint32 const along free dims
                return cu(val).to_broadcast([P] + shape_free)


            for ti, (jb, cn) in enumerate(col_tiles):
                pts_t = sb.tile([P, cn, 3], dt.float32, tag="pts")
                nc.sync.dma_start(out=pts_t[:, :, :], in_=pts_v[:, jb:jb + cn, :])

                # xn = clip((pts+1)*0.5, 0, 1)
                xn = sb.tile([P, cn, 3], dt.float32, tag="xn")
                nc.vector.tensor_scalar(
                    out=xn[:, :, :], in0=pts_t[:, :, :], scalar1=0.5, scalar2=0.5,
                    op0=Alu.mult, op1=Alu.add)
                nc.vector.tensor_scalar(
                    out=xn[:, :, :], in0=xn[:, :, :], scalar1=1.0, scalar2=0.0,
                    op0=Alu.min, op1=Alu.max)

                outt = sb.tile([P, cn, nout], dt.float32, tag="outt")

                for li, l in enumerate(levels):
                    res = H * (2 ** l)
                    size = OFFSETS[l + 1] - OFFSETS[l]
                    dense = (res + 1) ** D <= size

                    pos3 = sb.tile([P, cn, 3], dt.float32, tag="pos3")
                    nc.vector.tensor_scalar(
                        out=pos3[:, :, :], in0=xn[:, :, :], scalar1=float(res),
                        scalar2=None, op0=Alu.mult)

                    # floor: r = rint(pos); rf = f32(r); gt = (rf > pos); pgf = rf-gt
                    pgu = sb.tile([P, cn, 3], dt.uint32, tag="pgu")
                    rf = sb.tile([P, cn, 3], dt.float32, tag="rf")
                    gt = sb.tile([P, cn, 3], dt.float32, tag="gtf")
                    nc.vector.tensor_copy(out=pgu[:, :, :], in_=pos3[:, :, :])
                    nc.vector.tensor_copy(out=rf[:, :, :], in_=pgu[:, :, :])
                    nc.vector.tensor_tensor(
                        out=gt[:, :, :], in0=rf[:, :, :], in1=pos3[:, :, :],
                        op=Alu.is_gt)
                    nc.vector.tensor_tensor(
                        out=rf[:, :, :], in0=rf[:, :, :], in1=gt[:, :, :],
                        op=Alu.subtract)
                    nc.vector.tensor_scalar(
                        out=rf[:, :, :], in0=rf[:, :, :], scalar1=float(res - 1),
                        scalar2=None, op0=Alu.min)
                    # f2[0]=1-frac, f2[1]=frac ; frac = pos - pgf
                    f2 = sb.tile([P, 2, 3, cn], dt.float32, tag="f2")
                    frac = sb.tile([P, cn, 3], dt.float32, tag="frac")
                    nc.vector.tensor_tensor(
                        out=frac[:, :, :], in0=pos3[:, :, :], in1=rf[:, :, :],
                        op=Alu.subtract)
                    nc.vector.tensor_copy(out=pgu[:, :, :], in_=rf[:, :, :])
                    for d in range(3):
                        nc.vector.tensor_copy(
                            out=f2[:, 1, d, :], in_=frac[:, :, d])
                        # (frac * -1) - (-1) = 1 - frac
                        nc.vector.tensor_scalar(
                            out=f2[:, 0, d, :], in0=frac[:, :, d], scalar1=-1.0,
                            scalar2=-1.0, op0=Alu.mult, op1=Alu.subtract)

                    # ---- corner term pairs trm[d][0/1]: [P, cn] uint32 ----
                    trm = sb.tile([P, 3, 2, cn], dt.uint32, tag="trm")
                    nc.vector.tensor_copy(out=trm[:, 0, 0, :], in_=pgu[:, :, 0])
                    nc.vector.tensor_tensor(
                        out=trm[:, 0, 1, :], in0=pgu[:, :, 0], in1=ibc(1, [cn]),
                        op=Alu.add)
                    if dense:
                        s1, s2 = res + 1, (res + 1) ** 2
                        for d, s in ((1, s1), (2, s2)):
                            nc.vector.tensor_tensor(
                                out=trm[:, d, 0, :], in0=pgu[:, :, d],
                                in1=ibc(s, [cn]), op=Alu.mult)
                            nc.vector.tensor_tensor(
                                out=trm[:, d, 1, :], in0=trm[:, d, 0, :],
                                in1=ibc(s, [cn]), op=Alu.add)
                    else:
                        # y*p mod 2^19 via 5-bit chunks: products < 2^24 stay
                        # exact through the DVE's float multiply path.
                        nbits = l + 5
                        nch = -(-nbits // 5)
                        for d in (1, 2):
                            p = PRIMES[d]
                            acc = None
                            for jc in range(nch):
                                pk = (p << (5 * jc)) & MASK19
                                nib = sb.tile([P, cn], dt.uint32, tag="nib")
                                if jc == 0:
                                    nc.vector.tensor_scalar(
                                        out=nib[:, :], in0=pgu[:, :, d],
                                        scalar1=cu(31), scalar2=None,
                                        op0=Alu.bitwise_and)
                                else:
                                    nc.vector.tensor_tensor(
                                        out=nib[:, :], in0=pgu[:, :, d],
                                        in1=ibc(5 * jc, [cn]),
                                        op=Alu.logical_shift_right)
                                    nc.vector.tensor_scalar(
                                        out=nib[:, :], in0=nib[:, :],
                                        scalar1=cu(31), scalar2=None,
                                        op0=Alu.bitwise_and)
                                nc.vector.tensor_tensor(
                                    out=nib[:, :], in0=nib[:, :],
                                    in1=ibc(pk, [cn]), op=Alu.mult)
                                nc.vector.tensor_scalar(
                                    out=nib[:, :], in0=nib[:, :],
                                    scalar1=cu(MASK19), scalar2=None,
                                    op0=Alu.bitwise_and)
                                if acc is None:
                                    acc = sb.tile([P, cn], dt.uint32, tag="hacc")
                                    nc.vector.tensor_copy(out=acc[:, :], in_=nib[:, :])
                                else:
                                    nc.vector.tensor_tensor(
                                        out=acc[:, :], in0=acc[:, :],
                                        in1=nib[:, :], op=Alu.add)
                            nc.vector.tensor_copy(out=trm[:, d, 0, :], in_=acc[:, :])
                            nc.vector.tensor_tensor(
                                out=trm[:, d, 1, :], in0=trm[:, d, 0, :],
                                in1=ibc(p & MASK19, [cn]), op=Alu.add)

                    # ---- combine to 8 corner indices (k = bx*4 + by*2 + bz) ----
                    comb_op = Alu.add if dense else Alu.bitwise_xor
                    trm_f = trm[:, :, :, :]
                    part = trm_f.ap[0]
                    xy = sb.tile([P, 2, 2, cn], dt.uint32, tag="xy")
                    in_x = bass.AP(trm_f.tensor, trm[:, 0, 0, :].offset,
                                   [part, [cn, 2], [0, 2], [1, cn]])
                    in_y = bass.AP(trm_f.tensor, trm[:, 1, 0, :].offset,
                                   [part, [0, 2], [cn, 2], [1, cn]])
                    nc.vector.tensor_tensor(
                        out=xy[:, :, :, :], in0=in_x, in1=in_y, op=comb_op)
                    idx8 = sb.tile([P, 8, cn], dt.uint32, tag="idx8")
                    xy_f = xy[:, :, :, :]
                    idx8_f = idx8[:, :, :]
                    for bz in range(2):
                        in_xy = bass.AP(xy_f.tensor, xy_f.offset,
                                        [xy_f.ap[0], [2 * cn, 2], [cn, 2],
                                         [1, cn]])
                        in_z = bass.AP(trm_f.tensor,
                                       trm[:, 2, bz, :].offset,
                                       [part, [0, 2], [0, 2], [1, cn]])
                        o_z = bass.AP(idx8_f.tensor, idx8_f.offset + bz * cn,
                                      [idx8_f.ap[0], [4 * cn, 2], [2 * cn, 2],
                                       [1, cn]])
                        nc.vector.tensor_tensor(
                            out=o_z, in0=in_xy, in1=in_z, op=comb_op)
                    if not dense:
                        nc.vector.tensor_scalar(
                            out=idx8[:, :, :], in0=idx8[:, :, :],
                            scalar1=cu(MASK19), scalar2=None,
                            op0=Alu.bitwise_and)
                    nc.vector.tensor_tensor(
                        out=idx8[:, :, :], in0=idx8[:, :, :],
                        in1=ibc(OFFSETS[l], [8, cn]), op=Alu.add)

                    # ---- gathers: one [128,1]-offset indirect DMA per (corner, col)
                    idx8_i = idx8[:, :, :].bitcast(dt.int32)
                    if dense:
                        # corners (x, x+1) are consecutive rows: fetch both
                        # (16B) per offset. Layout [P, yz, cn, (x, c)].
                        feats_dn = sb.tile([P, 4, cn, 4], dt.float32,
                                           tag="featsd")
                        for k in range(4):
                            for j in range(cn):
                                nc.gpsimd.indirect_dma_start(
                                    out=feats_dn[:, k, j, :],
                                    out_offset=None,
                                    in_=emb_d[:, :],
                                    in_offset=bass.IndirectOffsetOnAxis(
                                        ap=idx8_i[:, k, j:j + 1], axis=0),
                                )
                    else:
                        feats = sb.tile([P, 8, cn, 2], dt.float32, tag="feats")
                        for k in range(8):
                            for j in range(cn):
                                nc.gpsimd.indirect_dma_start(
                                    out=feats[:, k, j, :],
                                    out_offset=None,
                                    in_=emb_d[:, :],
                                    in_offset=bass.IndirectOffsetOnAxis(
                                        ap=idx8_i[:, k, j:j + 1], axis=0),
                                )

                    # ---- weights: w8[k] = fx_bx * fy_by * fz_bz ----
                    f2_f = f2[:, :, :, :]
                    xyw = sb.tile([P, 2, 2, cn], dt.float32, tag="xyw")
                    wx = bass.AP(f2_f.tensor, f2[:, 0, 0, :].offset,
                                 [f2_f.ap[0], [3 * cn, 2], [0, 2], [1, cn]])
                    wy = bass.AP(f2_f.tensor, f2[:, 0, 1, :].offset,
                                 [f2_f.ap[0], [0, 2], [3 * cn, 2], [1, cn]])
                    nc.vector.tensor_tensor(
                        out=xyw[:, :, :, :], in0=wx, in1=wy, op=Alu.mult)
                    w8 = sb.tile([P, 8, cn], dt.float32, tag="w8")
                    xyw_f = xyw[:, :, :, :]
                    w8_f = w8[:, :, :]
                    for bz in range(2):
                        in_xyw = bass.AP(xyw_f.tensor, xyw_f.offset,
                                         [xyw_f.ap[0], [2 * cn, 2], [cn, 2],
                                          [1, cn]])
                        wz = bass.AP(f2_f.tensor,
                                     f2[:, bz, 2, :].offset,
                                     [f2_f.ap[0], [0, 2], [0, 2], [1, cn]])
                        o_w = bass.AP(w8_f.tensor, w8_f.offset + bz * cn,
                                      [w8_f.ap[0], [4 * cn, 2], [2 * cn, 2],
                                       [1, cn]])
                        nc.vector.tensor_tensor(out=o_w, in0=in_xyw, in1=wz,
                                                op=Alu.mult)

                    # ---- interp: per channel, prod = w8*feats_c ; reduce over k
                    outt_f = outt[:, :, :]
                    for c in range(2):
                        prod = sb.tile([P, cn, 8], dt.float32, tag="prod")
                        if dense:
                            fd_f = feats_dn[:, :, :, :]
                            for bx in range(2):
                                w_v = bass.AP(w8_f.tensor,
                                              w8_f.offset + bx * 4 * cn,
                                              [w8_f.ap[0], [1, cn], [cn, 4]])
                                f_v = bass.AP(fd_f.tensor,
                                              fd_f.offset + bx * 2 + c,
                                              [fd_f.ap[0], [4, cn], [4 * cn, 4]])
                                o_v = bass.AP(prod[:, :, :].tensor,
                                              prod[:, :, :].offset + bx * 4,
                                              [prod[:, :, :].ap[0], [8, cn],
                                               [1, 4]])
                                nc.vector.tensor_tensor(
                                    out=o_v, in0=w_v, in1=f_v, op=Alu.mult)
                        else:
                            feats_f = feats[:, :, :, :]
                            w_v = bass.AP(w8_f.tensor, w8_f.offset,
                                          [w8_f.ap[0], [1, cn], [cn, 8]])
                            f_v = bass.AP(feats_f.tensor, feats_f.offset + c,
                                          [feats_f.ap[0], [2, cn], [2 * cn, 8]])
                            nc.vector.tensor_tensor(
                                out=prod[:, :, :], in0=w_v, in1=f_v, op=Alu.mult)
                        res_v = bass.AP(outt_f.tensor,
                                        outt_f.offset + li * 2 + c,
                                        [outt_f.ap[0], [nout, cn]])
                        nc.vector.tensor_reduce(
                            out=res_v, in_=prod[:, :, :],
                            axis=mybir.AxisListType.X, op=Alu.add)

                nc.sync.dma_start(out=out_v[:, jb:jb + cn, :], in_=outt[:, :, :])

    nc.compile()
    return nc


_BUILD_CACHE = {}


def _get_nc(npc, nc_cols, cn_tile, levels):
    key = (npc, nc_cols, cn_tile, tuple(levels))
    if key not in _BUILD_CACHE:
        _BUILD_CACHE[key] = _build(npc, nc_cols, cn_tile, levels)
    return _BUILD_CACHE[key]


def kernel(inputs: np.ndarray, embeddings: np.ndarray, _trace=False) -> np.ndarray:
    from concourse.bass_utils import run_bass_kernel_spmd

    inputs = np.ascontiguousarray(inputs, dtype=np.float32)
    embeddings = np.ascontiguousarray(embeddings, dtype=np.float32)
    B = inputs.shape[0]

    pts_pad = np.zeros((B_PAD, 3), dtype=np.float32)
    pts_pad[:B] = inputs
    nc = _get_nc(NPC, NC_COLS, CN_TILE, list(range(L)))
    in_maps = [dict(pts=pts_pad[c * NPC:(c + 1) * NPC], emb=embeddings)
               for c in range(N_CORES)]
    import time as _time
    _t0 = _time.time()
    r = run_bass_kernel_spmd(nc, in_maps, core_ids=list(range(N_CORES)),
                             trace=False)
    kernel._last_wall_s = _time.time() - _t0
    out = np.concatenate([r.results[c]["out"] for c in range(N_CORES)], axis=0)
    kernel._last_exec_ns = r.exec_time_ns
    return out[:B]
